# revision 1
# baseline (speedup 1.0000x reference)
"""Distributed Trainium2 kernel for nn_AdaptivePooling (sliding-window
mean/max/logvar pooling + linear projection).

Reference computation (B=64, D=256, T=4096, kernel=16, stride=8, N=511):
    win[b,d,n,:] = x[b, d, 8n : 8n+16]
    pooled = w0*mean(win) + w1*max(win) + w2*log(clip(var_unbiased(win)))
    out[b,e,n] = sum_d proj_w[e,d] * pooled[b,d,n] + proj_b[e]
with [w0,w1,w2] = softmax(pool_weights).

Strategy: data-parallel over batch across 8 NeuronCores (8 batches/core).

v3 design (centered variance, fp16 streams, k-major layout):
  * The input contains near-degenerate windows (15*var16 down to 7e-6),
    so the variance MUST be computed in centered form: DEV = x - mean8
    keeps full relative precision; one-pass ssq-sum^2 cancels
    catastrophically there (measured 0.15 rel err vs 8e-4 centered).
  * x is stored in HBM as fp16 AND k-major transposed ([BL, D, 8, C],
    t = 8c+k): halves DMA traffic, and puts the chunk axis innermost so
    the per-chunk mean broadcast-subtract is a packed 16-bit 2x
    TensorTensor instead of a 1x scan (4.3us vs 8.6us per batch).
  * Chan combine of two 8-chunks per 16-window (all terms >= 0):
        q = m2c8[n] + m2c8[n+1] + (2*(mean8[n]-mean8[n+1]))^2
          = 15 * var_unbiased16
    logvar = Ln(q/15 + 1e-6): the +1e-6 activation bias replaces the
    reference's clip (max error ln2 at the clip floor, NaN-proof).
  * Engine balance per batch tile [128 part, 2 halves, 8 k, 512 c]:
      PE   : sum8 + m2c8 via identity-matmul accumulation over k-slices
             into PSUM (f32), then the projection
      DVE  : DEV broadcast-subtract (2x), SQD h1 (2x), max8 fold tree
             (2x), shifted-window combines (all fp16 2x)
      ACT  : SQD h0 (Square), PSUM evacuations, (2*dA)^2, Ln, out bias
      DMA  : fp16 in (2.1 MB/batch) + fp16 out (0.26 MB/batch)
  * Projection folds softmax weights into host-prefolded fp16 weights:
      Wcat = [w0/2*W | w1*W | w2*W], rhs = [sum16/8; max16; logvar]
"""

import numpy as np

B, D, T = 64, 256, 4096
KER, STR = 16, 8
N = (T - KER) // STR + 1  # 511
C = T // STR  # 512 chunks
N_CORES = 8
BL = B // N_CORES  # 8 batches per core

_CACHE: dict = {}


def _build(reps=1, max_on_pool=True, ssq_h1_tree=True, sq_h0_act=True,
           proj_delay=True):
    from concourse import bacc, mybir, tile

    F32 = mybir.dt.float32
    F16 = mybir.dt.float16
    ALU = mybir.AluOpType
    ACT = mybir.ActivationFunctionType
    AX = mybir.AxisListType.X

    nc = bacc.Bacc("TRN2", target_bir_lowering=False, debug=False,
                   num_devices=N_CORES)
    x_ext = nc.dram_tensor("x", [BL, D, 8, C], F16, kind="ExternalInput").ap()
    wt_ext = nc.dram_tensor("wt", [128, 6, 256], F16, kind="ExternalInput").ap()
    eye_ext = nc.dram_tensor("eye", [128, 128], F16, kind="ExternalInput").ap()
    out_ext = nc.dram_tensor("out", [BL, D, N], F16, kind="ExternalOutput").ap()

    with tile.TileContext(nc) as tc:
        with (
            tc.tile_pool(name="wpool", bufs=1) as wpool,
            tc.tile_pool(name="xpool", bufs=3) as xpool,
            tc.tile_pool(name="sqp", bufs=2) as sqp,
            tc.tile_pool(name="treep", bufs=2) as treep,
            tc.tile_pool(name="r8", bufs=3) as r8,
            tc.tile_pool(name="stp", bufs=3) as stp,
            tc.tile_pool(name="tmpp", bufs=2) as tmpp,
            tc.tile_pool(name="opool", bufs=4) as opool,
            tc.tile_pool(name="ps_s", bufs=1, space="PSUM") as ps_sp,
            tc.tile_pool(name="ps_q", bufs=1, space="PSUM") as ps_qp,
            tc.tile_pool(name="ps_o", bufs=2, space="PSUM") as ps_op,
        ):
            wt = wpool.tile([128, 6, 256], F16)
            nc.sync.dma_start(wt[:], wt_ext[:])
            eye = wpool.tile([128, 128], F16)
            nc.sync.dma_start(eye[:], eye_ext[:])

            rep_ctx = tc.For_i(0, reps, 1) if reps > 1 else None
            if rep_ctx is not None:
                rep_ctx.__enter__()

            pend = None       # (A, st, b) awaiting projection (depth 2)
            pend_late = None  # closure finishing batch b-1's q/logvar

            def emit_proj(A, st, b):
                # mean term is linear: project A (mean8) twice, shifted, so
                # sum16/8 never materializes (weights carry w0/2 * 1/2 each
                # via the host fold of w0/2 applied to A[n] + A[n+1] = 2*st0)
                ps = ps_op.tile([128, 2, 512], F32, tag="o")
                for eh in range(2):
                    k = 0
                    for h in range(2):
                        nc.tensor.matmul(
                            ps[:, eh, 0:N],
                            wt[:, h, eh * 128:(eh + 1) * 128],
                            A[:, h, 0:N],
                            start=(k == 0), stop=False)
                        k += 1
                        nc.tensor.matmul(
                            ps[:, eh, 0:N],
                            wt[:, h, eh * 128:(eh + 1) * 128],
                            A[:, h, 1:C],
                            start=False, stop=False)
                        k += 1
                    for s in range(1, 3):
                        for h in range(2):
                            nc.tensor.matmul(
                                ps[:, eh, 0:N],
                                wt[:, s * 2 + h, eh * 128:(eh + 1) * 128],
                                st[:, h, s - 1, :],
                                start=False, stop=(k == 7))
                            k += 1
                ob = opool.tile([128, 2, N], F16, tag="ob")
                nc.scalar.activation(ob[:], ps[:, :, 0:N], ACT.Identity)
                nc.sync.dma_start(
                    out_ext[b].rearrange("(eh p) n -> p eh n", p=128), ob[:])

            for b in range(BL):
                # k-major layout: x_ext[b] is [D, 8, C], t = 8c + k
                X = xpool.tile([128, 2, 8, C], F16, tag="x")
                nc.sync.dma_start(
                    X[:], x_ext[b].rearrange("(h p) k c -> p h k c", p=128))

                # --- chunk sums on PE -> mean8 (fp16) ---
                ps_s = ps_sp.tile([128, 2, C], F32, tag="s")
                for h in range(2):
                    for k in range(8):
                        nc.tensor.matmul(ps_s[:, h, :], eye[:], X[:, h, k, :],
                                         start=(k == 0), stop=(k == 7))
                A = r8.tile([128, 2, C], F16, tag="A")  # mean8 = sum8/8
                nc.scalar.activation(A[:], ps_s[:], ACT.Identity, scale=0.125)

                # finish batch b-1 smalls, then project batch b-2:
                # fills PE's wait on SQD and keeps DVE/ACT fed
                if pend is not None:
                    emit_proj(*pend)
                    pend = None
                if pend_late is not None:
                    pend = pend_late()
                    pend_late = None

                # --- centered deviations (broadcast over k, fp16 2x) ---
                DEV = sqp.tile([128, 2, 8, C], F16, tag="dev")
                Ab = A[:].rearrange("p h (o c) -> p h o c", o=1) \
                         .broadcast_to([128, 2, 8, C])
                nc.vector.tensor_tensor(DEV[:], X[:], Ab, op=ALU.subtract)
                # squares + second moments per half: PE starts after half 0
                SQD = sqp.tile([128, 2, 8, C], F16, tag="sqd")
                ps_q = ps_qp.tile([128, 2, C], F32, tag="q")
                for h in range(2):
                    nc.scalar.activation(SQD[:, h], DEV[:, h], ACT.Square)
                    for k in range(8):
                        nc.tensor.matmul(ps_q[:, h, :], eye[:],
                                         SQD[:, h, k, :],
                                         start=(k == 0), stop=(k == 7))
                m2c8 = r8.tile([128, 2, C], F16, tag="m2c8")
                nc.scalar.activation(m2c8[:], ps_q[:], ACT.Identity,
                                     scale=0.25)

                # --- chunk max: fold tree over k (fp16 2x) ---
                max8 = r8.tile([128, 2, C], F16, tag="max8")
                M1 = treep.tile([128, 2, 4, C], F16, tag="m1")
                nc.vector.tensor_tensor(
                    M1[:], X[:, :, 0:4, :], X[:, :, 4:8, :], op=ALU.max)
                M2 = treep.tile([128, 2, 2, C], F16, tag="m2")
                nc.vector.tensor_tensor(
                    M2[:], M1[:, :, 0:2, :], M1[:, :, 2:4, :], op=ALU.max)
                nc.vector.tensor_tensor(
                    max8[:], M2[:, :, 0, :], M2[:, :, 1, :], op=ALU.max)

                # --- window (16) stats; st = [max16, logvar] ---
                st = stp.tile([128, 2, 2, N], F16, tag="st")
                dA = tmpp.tile([128, 2, N], F16, tag="dA")
                nc.vector.tensor_sub(
                    dA[:], A[:, :, 0:N], A[:, :, 1:C])
                nc.vector.tensor_tensor(
                    st[:, :, 0, :], max8[:, :, 0:N], max8[:, :, 1:C],
                    op=ALU.max)

                def late(A=A, st=st, dA=dA, m2c8=m2c8, b=b):
                    # q/4 = m2c16/4 + dA^2 (m2c8 evacuated at /4; the *4
                    # refolds into the Ln scale); dA^2 is a plain DVE mult
                    dAsq = tmpp.tile([128, 2, N], F16, tag="dAsq")
                    nc.vector.tensor_tensor(
                        dAsq[:], dA[:], dA[:], op=ALU.mult)
                    m2c16 = tmpp.tile([128, 2, N], F16, tag="m2c16")
                    nc.vector.tensor_add(
                        m2c16[:], m2c8[:, :, 0:N], m2c8[:, :, 1:C])
                    q = tmpp.tile([128, 2, N], F16, tag="qq")
                    nc.vector.tensor_tensor(
                        q[:], m2c16[:], dAsq[:], op=ALU.add)
                    nc.vector.tensor_scalar(
                        q[:], q[:], 3.75e-6, None, op0=ALU.max)
                    nc.scalar.activation(st[:, :, 1, :], q[:], ACT.Ln,
                                         scale=4.0 / 15.0)
                    return (A, st, b)

                pend_late = late

            if pend is not None:
                emit_proj(*pend)
            if pend_late is not None:
                emit_proj(*pend_late())

            if rep_ctx is not None:
                rep_ctx.__exit__(None, None, None)

    nc.compile()
    return nc


def _get_nc():
    if "nc" not in _CACHE:
        _CACHE["nc"] = _build()
    return _CACHE["nc"]


def _prep_host(pool_weights, proj_w, proj_b):
    pw = np.asarray(pool_weights, np.float32)
    e = np.exp(pw - pw.max())
    w = (e / e.sum()).astype(np.float32)

    W = np.asarray(proj_w, np.float32)  # [E, D]
    # st0 carries sum16/8, so the mean weight folds w0/2
    Wcat = np.concatenate(
        [(w[0] / 2.0) * W, w[1] * W, w[2] * W], axis=1)  # [256, 768]
    lhsT = np.ascontiguousarray(Wcat.T)  # [768, 256]
    wt_host = np.ascontiguousarray(
        lhsT.reshape(6, 128, 256).transpose(1, 0, 2)).astype(np.float16)
    eye = np.eye(128, dtype=np.float16)
    return wt_host, eye


def _make_in_maps(x, pool_weights, proj_w, proj_b):
    wt_host, eye = _prep_host(pool_weights, proj_w, proj_b)
    # k-major: [B, D, T] -> [B, D, 8, C] with t = 8c + k
    x_h = np.ascontiguousarray(
        np.asarray(x).astype(np.float16)
        .reshape(B, D, C, 8).transpose(0, 1, 3, 2))
    return [
        {"x": x_h[i * BL:(i + 1) * BL], "wt": wt_host, "eye": eye}
        for i in range(N_CORES)
    ]


def _get_runner():
    """Cached jitted SPMD runner (avoids re-tracing the PJRT wrapper on
    every kernel() call).  Mirrors bass2jax.run_bass_via_pjrt."""
    if "runner" in _CACHE:
        return _CACHE["runner"]

    import jax
    from concourse import mybir
    from concourse.bass2jax import (
        _bass_exec_p, install_neuronx_cc_hook, partition_id_tensor)
    from jax.sharding import Mesh, PartitionSpec
    from jax.experimental.shard_map import shard_map

    nc = _get_nc()
    install_neuronx_cc_hook()

    partition_name = (nc.partition_id_tensor.name
                      if nc.partition_id_tensor else None)
    in_names, out_names, out_avals, zero_shapes = [], [], [], []
    for alloc in nc.m.functions[0].allocations:
        if not isinstance(alloc, mybir.MemoryLocationSet):
            continue
        name = alloc.memorylocations[0].name
        if alloc.kind == "ExternalInput":
            if name != partition_name:
                in_names.append(name)
        elif alloc.kind == "ExternalOutput":
            out_names.append(name)
            shape = tuple(alloc.tensor_shape)
            dtype = mybir.dt.np(alloc.dtype)
            out_avals.append(jax.core.ShapedArray(shape, dtype))
            zero_shapes.append((shape, dtype))
    n_params = len(in_names)
    n_outs = len(out_avals)
    all_in = in_names + out_names + ([partition_name] if partition_name else [])

    def _body(*args):
        operands = list(args)
        if partition_name is not None:
            operands.append(partition_id_tensor())
        outs = _bass_exec_p.bind(
            *operands, out_avals=tuple(out_avals), in_names=tuple(all_in),
            out_names=tuple(out_names), lowering_input_output_aliases=(),
            sim_require_finite=True, sim_require_nnan=True, nc=nc)
        return tuple(outs)

    devices = jax.devices()[:N_CORES]
    mesh = Mesh(np.asarray(devices), ("core",))
    in_specs = (PartitionSpec("core"),) * (n_params + n_outs)
    out_specs = (PartitionSpec("core"),) * n_outs
    donate = tuple(range(n_params, n_params + n_outs))
    sharded = jax.jit(
        shard_map(_body, mesh=mesh, in_specs=in_specs, out_specs=out_specs,
                  check_rep=False),
        donate_argnums=donate, keep_unused=True)
    sharding = jax.sharding.NamedSharding(mesh, PartitionSpec("core"))

    def run(in_maps):
        concat_in = [
            np.concatenate(
                [np.asarray(in_maps[c][nm]) for c in range(N_CORES)], axis=0)
            for nm in in_names
        ]
        dev_in = [jax.device_put(a, sharding) for a in concat_in]
        zs = [
            jax.device_put(
                np.zeros((N_CORES * s[0], *s[1:]), dt), sharding)
            for (s, dt) in zero_shapes
        ]
        outs = sharded(*dev_in, *zs)
        return {
            nm: np.asarray(outs[i]).reshape(N_CORES, *out_avals[i].shape)
            for i, nm in enumerate(out_names)
        }

    _CACHE["runner"] = run
    return run


def kernel(x, pool_weights, proj_w, proj_b):
    in_maps = _make_in_maps(x, pool_weights, proj_w, proj_b)
    res = _get_runner()(in_maps)
    out = res["out"].reshape(B, D, N).astype(np.float32)
    pb = np.asarray(proj_b, np.float32)
    if np.any(pb):
        out = out + pb[None, :, None]
    return np.ascontiguousarray(out)

